# revision 5
# baseline (speedup 1.0000x reference)
"""RWKV-4 WKV attention layer on 8 TRN2 NeuronCores — v2 (restructured).

Distribution (vs baseline):
  - T-shard: core i owns tokens [512i, 512(i+1)); for the scan core i owns
    global channel ptiles {i, i+8}, so A2A half h carries ptiles [8h, 8h+8)
    == one 128-row block per rank, and both halves fire at the halfway point
    of a strip-sequential weight stream (weights stream exactly once).
  - k and v projection passes are interleaved per weight strip so the scan's
    inputs finish early; their A2As fire per half.
  - r is NEVER exchanged: sigmoid(r) is consumed at (channel, my-token)
    coordinates which this core owns post-A2A#2.  r is drained raw to SBUF;
    sigmoid + multiply into the received y happens at phase C start.
  - Engine map (queues are in-order, so each phase owns distinct engines):
      PE    : matmuls only
      ACT   : kv weight-strip DMA triggers, all PSUM drains, scan exps
              (exp(k), exp(k+u) via per-partition bias), wo DMA triggers,
              sigmoids
      DVE   : time-mixes, r-strip s0/s1 DMA triggers, scans + num/den adds +
              reciprocal + y mul, atb assembly DMAs + sr*y muls, gate token
      GPSIMD: scan readback DMAs, ek*v / eku*v muls, carry copies,
              collectives
      SP    : x transposes, kv slab staging DMAs, r-strip s2/s3 triggers,
              y staging DMAs, out writes
  - Emission order: mixes | kv strips (A2A halves inside) | phase-B scan
    pt0,pt1 (y A2As after pt1's muls) | wo loads | r strips | phase C.
"""

import math
import os
import sys
from contextlib import ExitStack

for _p in ("/opt/trn_rl_repo", "/root/.axon_site/_ro/trn_rl_repo"):
    if os.path.isdir(_p) and _p not in sys.path:
        sys.path.insert(0, _p)

import numpy as np
import ml_dtypes

import concourse.bass as bass
import concourse.tile as tile
from concourse import bacc, mybir
from concourse.bass_utils import run_bass_kernel_spmd

F32 = mybir.dt.float32
BF16 = mybir.dt.bfloat16
F8 = mybir.dt.float8e4
AL = mybir.AluOpType
ACTF = mybir.ActivationFunctionType
P = 128


class Cfg:
    def __init__(self, T=4096, NE=2048, DA=2048, NC=8, TH=512):
        self.T, self.NE, self.DA, self.NC = T, NE, DA, NC
        self.TSL = T // NC          # tokens per core
        self.CSL = DA // NC         # channels per core
        self.NKT = NE // P          # contraction ptiles (projections)
        self.NMT = self.TSL // P    # token ptiles per slice
        self.NDT = DA // P          # output-channel ptiles (projections)
        self.NCT = self.CSL // P    # channel ptiles per core (2)
        self.NG = DA // 512         # weight strips per projection
        self.NKT2 = DA // P         # contraction ptiles (output matmul)
        self.NOT = NE // 512        # output strips (output matmul)
        self.TH = min(TH, T)        # scan chunk length
        self.NH = T // self.TH
        assert self.TSL % P == 0 and self.CSL % P == 0 and self.NCT == 2
        assert DA % 512 == 0 and NE % 512 == 0 and T % self.TH == 0


def _bcast(ap, n):
    """[P,1] AP -> [P,n] stride-0 broadcast along free."""
    return bass.AP(ap.tensor, ap.offset, [ap.ap[0], [0, n]])


def build_kernel(cfg: Cfg, no_cc: bool = False, reps: int = 1,
                 ablate: str | None = None):
    nc = bacc.Bacc("TRN2", target_bir_lowering=False, debug=False,
                   num_devices=1 if no_cc else cfg.NC)

    def _collective(ins, outs):
        if no_cc:
            nc.gpsimd.dma_start(out=outs[0], in_=ins[0])
        else:
            nc.gpsimd.collective_compute(
                "AllToAll", AL.bypass, replica_groups=[list(range(cfg.NC))],
                ins=ins, outs=outs)

    T, NE, DA, NC = cfg.T, cfg.NE, cfg.DA, cfg.NC
    TSL = cfg.TSL

    xs = nc.declare_dram_parameter("xs", [TSL + P, NE], BF16, isOutput=False)
    wk = nc.declare_dram_parameter("wk", [cfg.NG * P, cfg.NKT * 512], BF16, isOutput=False)
    wv = nc.declare_dram_parameter("wv", [cfg.NG * P, cfg.NKT * 512], BF16, isOutput=False)
    wr = nc.declare_dram_parameter("wr", [cfg.NG * P, cfg.NKT * 512], F8, isOutput=False)
    wo = nc.declare_dram_parameter("wo", [cfg.NOT * P, cfg.NKT2 * 512], BF16, isOutput=False)
    tmk = nc.declare_dram_parameter("tmk", [P, cfg.NKT], F32, isOutput=False)
    tmv = nc.declare_dram_parameter("tmv", [P, cfg.NKT], F32, isOutput=False)
    tmr = nc.declare_dram_parameter("tmr", [P, cfg.NKT], F32, isOutput=False)
    lam = nc.declare_dram_parameter("lam", [P, cfg.NCT], F32, isOutput=False)
    ub = nc.declare_dram_parameter("ub", [P, cfg.NCT], F32, isOutput=False)
    out = nc.declare_dram_parameter("out", [TSL, NE], F32, isOutput=True)

    with tile.TileContext(nc) as tc, ExitStack() as octx:
        dram = octx.enter_context(tc.tile_pool(name="dram", bufs=1, space="DRAM"))
        psum = octx.enter_context(tc.tile_pool(name="psum", bufs=8, space="PSUM"))
        const_pool = octx.enter_context(tc.tile_pool(name="const", bufs=1))
        tokp = octx.enter_context(tc.tile_pool(name="tokp", bufs=2))

        tm_sb = {}
        for name, src in (("k", tmk), ("v", tmv), ("r", tmr)):
            t = const_pool.tile([P, cfg.NKT], F32, tag=f"tm{name}")
            nc.sync.dma_start(t[:], src[:])
            tm_sb[name] = t
        lam_sb = const_pool.tile([P, cfg.NCT], F32, tag="lam")
        nc.sync.dma_start(lam_sb[:], lam[:])
        ub_sb = const_pool.tile([P, cfg.NCT], F32, tag="ub")
        nc.sync.dma_start(ub_sb[:], ub[:])

        # DRAM bounce buffers for collectives (shared across reps).
        HDA = NC * P
        a2a = {}
        for name in ("k", "v", "y"):
            a2a[name] = {
                "in": [dram.tile([HDA, TSL], BF16, tag=f"ai_{name}{h}",
                                 name=f"ai_{name}{h}") for h in range(cfg.NCT)],
                "out": [dram.tile([HDA, TSL], BF16, tag=f"ao_{name}{h}",
                                  name=f"ao_{name}{h}") for h in range(cfg.NCT)],
            }

        wdram = {"k": wk, "v": wv, "r": wr}
        prev_tok = None
        for rep in range(reps):
            prev_tok = _emit_body(nc, tc, cfg, rep, tm_sb, lam_sb, ub_sb,
                                  a2a, xs, wdram, wo, out, psum,
                                  _collective, tokp, prev_tok, ablate)

    nc.finalize()
    return nc


def _make_token(nc, tokp, osts, R):
    tok = tokp.tile([1, 8], F32, tag="tok", name=R + "tok")
    for i, o in enumerate(osts):
        nc.vector.tensor_copy(tok[0:1, 2 * (i % 4):2 * (i % 4) + 2],
                              o[0:1, 0:2])
    return tok


def _emit_body(nc, tc, cfg, rep, tm_sb, lam_sb, ub_sb, a2a,
               xs, wdram, wo, out, psum, _collective, tokp, prev_tok,
               ablate=None):
    T, NC, TSL, TH, NH = cfg.T, cfg.NC, cfg.TSL, cfg.TH, cfg.NH
    XW = TSL + P
    R = f"r{rep}_"

    with ExitStack() as body:
        # pools that span multiple phases (stack-allocated: LIFO only)
        srp = body.enter_context(tc.tile_pool(name=R + "srp", bufs=1))
        mixrp = body.enter_context(tc.tile_pool(name=R + "mixr", bufs=1))
        scanp = body.enter_context(tc.tile_pool(name=R + "scanp", bufs=2))
        carryp = body.enter_context(tc.tile_pool(name=R + "carryp", bufs=2))
        atbp = body.enter_context(tc.tile_pool(name=R + "atbp", bufs=1))

        srT = [srp.tile([P, TSL], BF16, tag=f"sr{dt}", name=R + f"sr{dt}")
               for dt in range(cfg.NDT)]
        mxr_big = mixrp.tile([P, cfg.NKT * TSL], F8, tag="mxr",
                             name=R + "mxr")
        mixes = {"r": [mxr_big[:, kt * TSL: (kt + 1) * TSL]
                       for kt in range(cfg.NKT)]}
        atb = atbp.tile([P, cfg.NKT2 * TSL], BF16, tag="atb", name=R + "atb")

        with tc.tile_pool(name=R + "mixkv", bufs=1) as mixkvp, \
             tc.tile_pool(name=R + "wstp", bufs=3) as wstp, \
             tc.tile_pool(name=R + "slabp", bufs=3) as slabp, \
             tc.tile_pool(name=R + "q0p", bufs=1) as q0p:
            # ---- step 0: transpose x (chunked), all time-mixes ------------
            with tc.tile_pool(name=R + "xtp", bufs=1) as xtp:
                xtrb = xtp.tile([P, cfg.NKT * XW], BF16, tag="xtrb",
                                name=R + "xtrb")
                if rep > 0:
                    nc.vector.tensor_copy(xtrb[0:1, 0:8], prev_tok[0:1, 0:8])
                for q in range(4):
                    kt0 = 4 * q
                    b = xtrb[:, XW * kt0: XW * (kt0 + 4)]
                    out3 = bass.AP(b.tensor, b.offset,
                                   [b.ap[0], [XW, 4], [1, XW]])
                    nc.sync.dma_start(out3, xs[:, P * kt0: P * (kt0 + 4)],
                                      transpose=True)
                for name in ("k", "v"):
                    mixes[name] = [mixkvp.tile([P, TSL], BF16,
                                               tag=f"mx{name}{kt}",
                                               name=R + f"mx{name}{kt}")
                                   for kt in range(cfg.NKT)]
                with tc.tile_pool(name=R + "dtp", bufs=cfg.NKT) as dtp:
                    dts = []
                    for kt in range(cfg.NKT):
                        xm = xtrb[:, XW * kt + P: XW * (kt + 1)]
                        xx = xtrb[:, XW * kt + P - 1: XW * (kt + 1) - 1]
                        d = dtp.tile([P, TSL], BF16, tag="d", name=R + f"d{kt}")
                        nc.vector.tensor_sub(d[:], xm, xx)
                        nc.vector.scalar_tensor_tensor(
                            mixes["k"][kt][:], d[:], tm_sb["k"][:, kt:kt + 1],
                            xx, op0=AL.mult, op1=AL.add)
                        dts.append((d, xx))
                    for name in ("v", "r"):
                        for kt in range(cfg.NKT):
                            d, xx = dts[kt]
                            mx = mixes[name][kt]
                            if not isinstance(mx, bass.AP):
                                mx = mx[:]
                            nc.vector.scalar_tensor_tensor(
                                mx, d[:],
                                tm_sb[name][:, kt:kt + 1], xx,
                                op0=AL.mult, op1=AL.add)
            # xtrb + d pools closed (space reusable once mixes have run)

            # ---- step 1: k/v projections, strip-interleaved ---------------
            def proj_group(name, dt, wtsl):
                pt_ = psum.tile([P, TSL], F32, tag="pp",
                                name=R + f"ps_{name}_{dt}")
                s4 = dt % 4
                for kt in range(cfg.NKT):
                    nc.tensor.matmul(
                        pt_[:], wtsl(kt, s4),
                        mixes[name][kt][:, :],
                        start=(kt == 0), stop=(kt == cfg.NKT - 1))
                if name == "r":
                    nc.scalar.copy(srT[dt][:], pt_[:])
                else:
                    slab = slabp.tile([P, TSL], BF16, tag="slab",
                                      name=R + f"sl_{name}_{dt}")
                    nc.scalar.copy(slab[:], pt_[:])
                    h, j = dt // 8, dt % 8
                    nc.sync.dma_start(
                        a2a[name]["in"][h][P * j: P * (j + 1), :], slab[:])

            order = [(name, g) for g in range(cfg.NG)
                     for name in ("k", "v")]
            loaded = {}

            def ensure(i):
                if 0 <= i < len(order) and i not in loaded:
                    name, g = order[i]
                    if i == 0:
                        qs = []
                        for q in range(4):
                            qt = q0p.tile([P, 2048], BF16, tag=f"q{q}",
                                          name=R + f"wq{q}")
                            nc.scalar.dma_start(
                                qt[:], wdram["k"][0:P,
                                                  2048 * q: 2048 * (q + 1)])
                            qs.append(qt)
                        loaded[i] = ("q", qs)
                    else:
                        wt = wstp.tile([P, cfg.NKT * 512], BF16, tag="wst",
                                       name=R + f"w_{name}_{g}")
                        nc.scalar.dma_start(
                            wt[:], wdram[name][P * g: P * (g + 1), :])
                        loaded[i] = ("s", wt)

            def mk_wtsl(entry):
                kind, w = entry
                if kind == "q":
                    return lambda kt, s4: w[kt // 4][
                        :, (kt % 4) * 512 + 128 * s4:
                        (kt % 4) * 512 + 128 * (s4 + 1)]
                return lambda kt, s4: w[
                    :, kt * 512 + 128 * s4: kt * 512 + 128 * (s4 + 1)]

            for i, (name, g) in enumerate(order):
                ensure(i), ensure(i + 1), ensure(i + 2)
                wtsl = mk_wtsl(loaded[i])
                for dt in range(4 * g, 4 * g + 4):
                    proj_group(name, dt, wtsl)
                if dt in (7, 15):
                    h = dt // 8
                    _collective([a2a[name]["in"][h][:].opt()],
                                [a2a[name]["out"][h][:].opt()])

            # ---- step 2: r strip loads (SP queue; wstp rotation) ----------
            rstrips = {}
            for g in range(cfg.NG):
                wt = wstp.tile([P, cfg.NKT * 512], F8, tag="wst",
                               name=R + f"w_r_{g}")
                nc.sync.dma_start(wt[:], wdram["r"][P * g: P * (g + 1), :])
                rstrips[g] = wt

            # wo prefetch on the SP queue (transfers run during the scan)
            wotile = [None] * cfg.NOT

            def load_wo(wop):
                for nt in range(cfg.NOT):
                    wot = wop.tile([P, cfg.NKT2 * 512], BF16, tag="wo",
                                   name=R + f"wo_{nt}")
                    nc.sync.dma_start(wot[:], wo[P * nt: P * (nt + 1), :])
                    wotile[nt] = wot

            # ---- step 3: phase B — WKV scan pt0 then pt1 ------------------
            # core's ptile p == global ptile 8p + rank, from A2A half p.
            for p in ([] if ablate == "A" else range(cfg.NCT)):
                lam_b = _bcast(lam_sb[:, p:p + 1], TH)
                ub_ap = ub_sb[:, p:p + 1]
                prevP = prevQ = None
                for c in range(NH):
                    sfx = f"_{p}_{c}"
                    BPC = TH // TSL        # token blocks per chunk
                    j0 = c * BPC

                    def rb(name):
                        t = scanp.tile([P, TH], BF16, tag=f"rb{name}",
                                       name=R + name + sfx)
                        s = a2a[name]["out"][p][:]
                        src = bass.AP(s.tensor, s.offset + j0 * P * TSL,
                                      [[TSL, P], [P * TSL, BPC], [1, TSL]])
                        dst = bass.AP(t[:, :].tensor, t[:, :].offset,
                                      [t[:, :].ap[0], [TSL, BPC], [1, TSL]])
                        nc.scalar.dma_start(dst, src)
                        return t

                    kc, vc = rb("k"), rb("v")
                    ek = scanp.tile([P, TH], BF16, tag="ek",
                                    name=R + "ek" + sfx)
                    nc.scalar.activation(ek[:], kc[:], ACTF.Exp)
                    eku = scanp.tile([P, TH], BF16, tag="eku",
                                     name=R + "eku" + sfx)
                    nc.scalar.activation(eku[:], kc[:], ACTF.Exp, bias=ub_ap)
                    ekv = scanp.tile([P, TH], BF16, tag="ekv",
                                     name=R + "ekv" + sfx)
                    nc.gpsimd.tensor_mul(ekv[:], ek[:], vc[:])
                    ekuv = scanp.tile([P, TH], BF16, tag="ekuv",
                                      name=R + "ekuv" + sfx)
                    nc.gpsimd.tensor_mul(ekuv[:], eku[:], vc[:])

                    Pst = carryp.tile([P, TH + 1], BF16, tag="Pst",
                                      name=R + "P" + sfx)
                    Qst = carryp.tile([P, TH + 1], BF16, tag="Qst",
                                      name=R + "Q" + sfx)
                    if c == 0:
                        nc.gpsimd.memset(Pst[:, 0:1], 0.0)
                        nc.gpsimd.memset(Qst[:, 0:1], 0.0)
                    else:
                        nc.gpsimd.tensor_copy(Pst[:, 0:1], prevP[:, TH:TH + 1])
                        nc.gpsimd.tensor_copy(Qst[:, 0:1], prevQ[:, TH:TH + 1])
                    nc.vector.tensor_tensor_scan(
                        Pst[:, 1:TH + 1], lam_b, ekv[:], Pst[:, 0:1],
                        op0=AL.mult, op1=AL.add)
                    nc.vector.tensor_tensor_scan(
                        Qst[:, 1:TH + 1], lam_b, ek[:], Qst[:, 0:1],
                        op0=AL.mult, op1=AL.add)

                    num = ekuv
                    nc.vector.tensor_add(num[:], ekuv[:], Pst[:, 0:TH])
                    den = scanp.tile([P, TH], F32, tag="den",
                                     name=R + "den" + sfx)
                    nc.vector.tensor_add(den[:], eku[:], Qst[:, 0:TH])
                    nc.vector.reciprocal_approx_fast(den[:], den[:])
                    y = kc    # kc dead once both exps have run; reuse as y
                    nc.vector.tensor_mul(y[:], num[:], den[:])

                    d = a2a["y"]["in"][p][:]
                    dst = bass.AP(d.tensor, d.offset + j0 * P * TSL,
                                  [[TSL, P], [P * TSL, BPC], [1, TSL]])
                    src = bass.AP(y[:, :].tensor, y[:, :].offset,
                                  [y[:, :].ap[0], [TSL, BPC], [1, TSL]])
                    nc.sync.dma_start(dst, src)
                    prevP, prevQ = Pst, Qst

            # ---- step 4: r projections (fp8 DoubleRow) + sigmoids ---------
            for g in range(cfg.NG):
                for dt in range(4 * g, 4 * g + 4):
                    pt_ = psum.tile([P, TSL], F32, tag="pp",
                                    name=R + f"ps_r_{dt}")
                    wt = rstrips[g][:, :]
                    mb = mxr_big[:, :]
                    s4 = dt % 4
                    for j in range(cfg.NKT // 2):
                        stat = bass.AP(wt.tensor,
                                       wt.offset + j * 1024 + s4 * 256,
                                       [wt.ap[0], [128, 2], [1, 128]])
                        mov = bass.AP(mb.tensor, mb.offset + 2 * j * TSL,
                                      [mb.ap[0], [TSL, 2], [1, TSL]])
                        nc.tensor.matmul(
                            pt_[:], stat, mov,
                            start=(j == 0), stop=(j == cfg.NKT // 2 - 1),
                            perf_mode=mybir.MatmulPerfMode.DoubleRow)
                    nc.scalar.copy(srT[dt][:], pt_[:])
                    nc.scalar.activation(srT[dt][:], srT[dt][:], ACTF.Sigmoid)
        # mixkv + wstp + slabp closed

        # ---- step 5: wo loads (gpsimd queue), y A2As + atb assembly -------
        with tc.tile_pool(name=R + "wop", bufs=4) as wop, \
             tc.tile_pool(name=R + "partp", bufs=16) as partp, \
             tc.tile_pool(name=R + "ostl", bufs=6) as ostl:
            load_wo(wop)

            def atb_half(h):
                for kt2 in range(8 * h, 8 * h + 8):
                    j = kt2 % 8
                    sl = atb[:, kt2 * TSL: (kt2 + 1) * TSL]
                    nc.gpsimd.dma_start(
                        sl, a2a["y"]["out"][h][P * j: P * (j + 1), :])

            _collective([a2a["y"]["in"][0][:].opt()],
                        [a2a["y"]["out"][0][:].opt()])
            atb_half(0)
            _collective([a2a["y"]["in"][1][:].opt()],
                        [a2a["y"]["out"][1][:].opt()])
            atb_half(1)

            # ---- step 6: phase C — sr*y muls (DVE) + output matmul --------
            for kt2 in range(cfg.NKT2):
                sl = atb[:, kt2 * TSL: (kt2 + 1) * TSL]
                nc.vector.tensor_mul(sl, sl, srT[kt2][:])

            if ablate == "B":
                lastc = []
                for mt in range(cfg.NMT):
                    oc = ostl.tile([P, 512], F32, tag="oc",
                                   name=R + f"abl_{mt}")
                    nc.scalar.copy(oc[:], atb[:, mt * 2048: mt * 2048 + 512])
                    nc.sync.dma_start(out[P * mt: P * (mt + 1), 0:512], oc[:])
                    lastc.append(oc)
                return _make_token(nc, tokp, lastc, R)

            # h0 contraction first (all four nt strips), bf16 partial
            # drains; h1 sweeps start once the second y half lands.
            parts = {}
            for rnd in range(cfg.NOT // 2):
                nts = (2 * rnd, 2 * rnd + 1)
                pts = {(mt_, i_): psum.tile([P, 512], F32, tag="pp",
                                            name=R + f"pa_{rnd}_{mt_}_{i_}")
                       for mt_ in range(cfg.NMT) for i_ in range(2)}
                for kt2 in range(8):
                    for mt in range(cfg.NMT):
                        lhsT = atb[:, kt2 * TSL + P * mt:
                                   kt2 * TSL + P * (mt + 1)]
                        for i_ in range(2):
                            nc.tensor.matmul(
                                pts[(mt, i_)][:], lhsT,
                                wotile[nts[i_]][:, 512 * kt2: 512 * (kt2 + 1)],
                                start=(kt2 == 0), stop=(kt2 == 7))
                for mt in range(cfg.NMT):
                    for i_ in range(2):
                        pb = partp.tile([P, 512], BF16, tag="part",
                                        name=R + f"pb_{rnd}_{mt}_{i_}")
                        nc.scalar.copy(pb[:], pts[(mt, i_)][:])
                        parts[(mt, nts[i_])] = pb

            lastc = []
            for rnd in range(cfg.NOT // 2):
                nts = (2 * rnd, 2 * rnd + 1)
                pts = {(mt_, i_): psum.tile([P, 512], F32, tag="pp",
                                            name=R + f"po_{rnd}_{mt_}_{i_}")
                       for mt_ in range(cfg.NMT) for i_ in range(2)}
                for kt2 in range(8, cfg.NKT2):
                    for mt in range(cfg.NMT):
                        lhsT = atb[:, kt2 * TSL + P * mt:
                                   kt2 * TSL + P * (mt + 1)]
                        for i_ in range(2):
                            nc.tensor.matmul(
                                pts[(mt, i_)][:], lhsT,
                                wotile[nts[i_]][:, 512 * kt2: 512 * (kt2 + 1)],
                                start=(kt2 == 8), stop=(kt2 == cfg.NKT2 - 1))
                for mt in range(cfg.NMT):
                    for i_ in range(2):
                        nt = nts[i_]
                        oc = ostl.tile([P, 512], F32, tag="oc",
                                       name=R + f"oc_{rnd}_{mt}_{i_}")
                        nc.vector.tensor_add(oc[:], parts[(mt, nt)][:],
                                             pts[(mt, i_)][:])
                        nc.sync.dma_start(
                            out[P * mt: P * (mt + 1),
                                512 * nt: 512 * (nt + 1)],
                            oc[:])
                        if rnd == cfg.NOT // 2 - 1 and i_ == 1:
                            lastc.append(oc)
            tok = _make_token(nc, tokp, lastc, R)
    return tok


# ------------------------------------------------------------------------
# host side
# ------------------------------------------------------------------------

_CACHE = {}


def _get_nc(cfg: Cfg):
    key = (cfg.T, cfg.NE, cfg.DA, cfg.NC, cfg.TH)
    if key not in _CACHE:
        _CACHE[key] = build_kernel(cfg)
    return _CACHE[key]


def make_in_maps(cfg: Cfg, x, time_first, time_decay, time_mix_k, time_mix_v,
                 time_mix_r, W_key, W_value, W_receptance, W_output):
    T, NE, DA, NC = cfg.T, cfg.NE, cfg.DA, cfg.NC
    TSL = cfg.TSL
    bf = ml_dtypes.bfloat16

    x = np.asarray(x, np.float32)
    xpad = np.zeros((P + T, NE), bf)
    xpad[P:] = x.astype(bf)

    def tile_w(w, nkt, ng):
        w = np.asarray(w, np.float32).astype(bf)
        return np.ascontiguousarray(
            w.reshape(nkt, P, ng, 512).transpose(2, 1, 0, 3)
            .reshape(ng * P, nkt * 512))

    wk16 = tile_w(W_key, cfg.NKT, cfg.NG)
    wv16 = tile_w(W_value, cfg.NKT, cfg.NG)
    wo16 = tile_w(W_output, cfg.NKT2, cfg.NOT)
    # r weights: fp8 e4m3 packed for DoubleRow —
    # [g*P+p, j*1024 + c4*256 + i*128 + m] = Wr[128*(2j+i)+p, 512g+128c4+m]
    f8np = mybir.dt.np(F8)
    wr4 = np.asarray(W_receptance, np.float32).astype(f8np) \
        .reshape(cfg.NKT // 2, 2, P, cfg.NG, 4, 128)
    wr8 = np.ascontiguousarray(
        wr4.transpose(3, 2, 0, 4, 1, 5).reshape(cfg.NG * P, cfg.NKT * 512))

    def col_fold(v, n_t):  # [n_t*P] -> [P, n_t]
        return np.ascontiguousarray(
            np.asarray(v, np.float64).reshape(-1)[: n_t * P]
            .reshape(n_t, P).T.astype(np.float32))

    tmk_a = col_fold(time_mix_k, cfg.NKT)
    tmv_a = col_fold(time_mix_v, cfg.NKT)
    tmr_a = col_fold(time_mix_r, cfg.NKT)

    td = np.asarray(time_decay, np.float64).reshape(-1)
    lam_full = np.exp(-np.exp(td)).astype(np.float32)
    ub_full = np.asarray(time_first, np.float32).reshape(-1)

    in_maps = []
    for i in range(NC):
        xsl = np.ascontiguousarray(xpad[TSL * i: TSL * i + TSL + P, :])
        # core i owns global channel ptiles {i, i+8}
        lam_i = np.stack([lam_full[P * (8 * p + i): P * (8 * p + i + 1)]
                          for p in range(cfg.NCT)], axis=1)
        ub_i = np.stack([ub_full[P * (8 * p + i): P * (8 * p + i + 1)]
                         for p in range(cfg.NCT)], axis=1)
        in_maps.append({
            "xs": xsl, "wk": wk16, "wv": wv16, "wr": wr8, "wo": wo16,
            "tmk": tmk_a, "tmv": tmv_a, "tmr": tmr_a,
            "lam": np.ascontiguousarray(lam_i),
            "ub": np.ascontiguousarray(ub_i),
        })
    return in_maps


def kernel(x, time_first, time_decay, time_mix_k, time_mix_v, time_mix_r,
           W_key, W_value, W_receptance, W_output, _trace=False):
    cfg = Cfg(T=int(np.asarray(x).shape[0]), NE=int(np.asarray(x).shape[1]),
              DA=int(np.asarray(time_decay).reshape(-1).shape[0]), NC=8)
    nc = _get_nc(cfg)
    in_maps = make_in_maps(cfg, x, time_first, time_decay, time_mix_k,
                           time_mix_v, time_mix_r, W_key, W_value,
                           W_receptance, W_output)
    res = run_bass_kernel_spmd(nc, in_maps, core_ids=list(range(cfg.NC)),
                               trace=_trace)
    outp = np.concatenate([res.results[i]["out"] for i in range(cfg.NC)], axis=0)
    out_final = outp.astype(np.float32)
    if _trace:
        return out_final, res
    return out_final


# revision 6
# speedup vs baseline: 1.0810x; 1.0810x over previous
"""RWKV-4 WKV attention layer on 8 TRN2 NeuronCores — v2 (restructured).

Distribution (vs baseline):
  - T-shard: core i owns tokens [512i, 512(i+1)); for the scan core i owns
    global channel ptiles {i, i+8}, so A2A half h carries ptiles [8h, 8h+8)
    == one 128-row block per rank, and both halves fire at the halfway point
    of a strip-sequential weight stream (weights stream exactly once).
  - k and v projection passes are interleaved per weight strip so the scan's
    inputs finish early; their A2As fire per half.
  - r is NEVER exchanged: sigmoid(r) is consumed at (channel, my-token)
    coordinates which this core owns post-A2A#2.  r is drained raw to SBUF;
    sigmoid + multiply into the received y happens at phase C start.
  - Engine map (queues are in-order, so each phase owns distinct engines):
      PE    : matmuls only
      ACT   : kv weight-strip DMA triggers, all PSUM drains, scan exps
              (exp(k), exp(k+u) via per-partition bias), wo DMA triggers,
              sigmoids
      DVE   : time-mixes, r-strip s0/s1 DMA triggers, scans + num/den adds +
              reciprocal + y mul, atb assembly DMAs + sr*y muls, gate token
      GPSIMD: scan readback DMAs, ek*v / eku*v muls, carry copies,
              collectives
      SP    : x transposes, kv slab staging DMAs, r-strip s2/s3 triggers,
              y staging DMAs, out writes
  - Emission order: mixes | kv strips (A2A halves inside) | phase-B scan
    pt0,pt1 (y A2As after pt1's muls) | wo loads | r strips | phase C.
"""

import math
import os
import sys
from contextlib import ExitStack

for _p in ("/opt/trn_rl_repo", "/root/.axon_site/_ro/trn_rl_repo"):
    if os.path.isdir(_p) and _p not in sys.path:
        sys.path.insert(0, _p)

import numpy as np
import ml_dtypes

import concourse.bass as bass
import concourse.tile as tile
from concourse import bacc, mybir
from concourse.bass_utils import run_bass_kernel_spmd

F32 = mybir.dt.float32
BF16 = mybir.dt.bfloat16
F8 = mybir.dt.float8e4
AL = mybir.AluOpType
ACTF = mybir.ActivationFunctionType
P = 128


class Cfg:
    def __init__(self, T=4096, NE=2048, DA=2048, NC=8, TH=1024):
        self.T, self.NE, self.DA, self.NC = T, NE, DA, NC
        self.TSL = T // NC          # tokens per core
        self.CSL = DA // NC         # channels per core
        self.NKT = NE // P          # contraction ptiles (projections)
        self.NMT = self.TSL // P    # token ptiles per slice
        self.NDT = DA // P          # output-channel ptiles (projections)
        self.NCT = self.CSL // P    # channel ptiles per core (2)
        self.NG = DA // 512         # weight strips per projection
        self.NKT2 = DA // P         # contraction ptiles (output matmul)
        self.NOT = NE // 512        # output strips (output matmul)
        self.TH = min(TH, T)        # scan chunk length
        self.NH = T // self.TH
        assert self.TSL % P == 0 and self.CSL % P == 0 and self.NCT == 2
        assert DA % 512 == 0 and NE % 512 == 0 and T % self.TH == 0


def _bcast(ap, n):
    """[P,1] AP -> [P,n] stride-0 broadcast along free."""
    return bass.AP(ap.tensor, ap.offset, [ap.ap[0], [0, n]])


def build_kernel(cfg: Cfg, no_cc: bool = False, reps: int = 1,
                 ablate: str | None = None):
    nc = bacc.Bacc("TRN2", target_bir_lowering=False, debug=False,
                   num_devices=1 if no_cc else cfg.NC)

    def _collective(ins, outs):
        if no_cc:
            nc.gpsimd.dma_start(out=outs[0], in_=ins[0])
        else:
            nc.gpsimd.collective_compute(
                "AllToAll", AL.bypass, replica_groups=[list(range(cfg.NC))],
                ins=ins, outs=outs)

    T, NE, DA, NC = cfg.T, cfg.NE, cfg.DA, cfg.NC
    TSL = cfg.TSL

    xs = nc.declare_dram_parameter("xs", [TSL + P, NE], BF16, isOutput=False)
    wk = nc.declare_dram_parameter("wk", [cfg.NG * P, cfg.NKT * 512], BF16, isOutput=False)
    wv = nc.declare_dram_parameter("wv", [cfg.NG * P, cfg.NKT * 512], BF16, isOutput=False)
    wr = nc.declare_dram_parameter("wr", [cfg.NG * P, cfg.NKT * 512], F8, isOutput=False)
    wo = nc.declare_dram_parameter("wo", [cfg.NOT * P, cfg.NKT2 * 512], BF16, isOutput=False)
    tmk = nc.declare_dram_parameter("tmk", [P, cfg.NKT], F32, isOutput=False)
    tmv = nc.declare_dram_parameter("tmv", [P, cfg.NKT], F32, isOutput=False)
    tmr = nc.declare_dram_parameter("tmr", [P, cfg.NKT], F32, isOutput=False)
    lam = nc.declare_dram_parameter("lam", [P, cfg.NCT], F32, isOutput=False)
    ub = nc.declare_dram_parameter("ub", [P, cfg.NCT], F32, isOutput=False)
    out = nc.declare_dram_parameter("out", [TSL, NE], F32, isOutput=True)

    with tile.TileContext(nc) as tc, ExitStack() as octx:
        dram = octx.enter_context(tc.tile_pool(name="dram", bufs=1, space="DRAM"))
        psum = octx.enter_context(tc.tile_pool(name="psum", bufs=8, space="PSUM"))
        const_pool = octx.enter_context(tc.tile_pool(name="const", bufs=1))
        tokp = octx.enter_context(tc.tile_pool(name="tokp", bufs=2))

        tm_sb = {}
        for name, src in (("k", tmk), ("v", tmv), ("r", tmr)):
            t = const_pool.tile([P, cfg.NKT], F32, tag=f"tm{name}")
            nc.sync.dma_start(t[:], src[:])
            tm_sb[name] = t
        lam_sb = const_pool.tile([P, cfg.NCT], F32, tag="lam")
        nc.sync.dma_start(lam_sb[:], lam[:])
        ub_sb = const_pool.tile([P, cfg.NCT], F32, tag="ub")
        nc.sync.dma_start(ub_sb[:], ub[:])

        # DRAM bounce buffers for collectives (shared across reps).
        HDA = NC * P
        a2a = {}
        for name in ("k", "v", "y"):
            a2a[name] = {
                "in": [dram.tile([HDA, TSL], BF16, tag=f"ai_{name}{h}",
                                 name=f"ai_{name}{h}") for h in range(cfg.NCT)],
                "out": [dram.tile([HDA, TSL], BF16, tag=f"ao_{name}{h}",
                                  name=f"ao_{name}{h}") for h in range(cfg.NCT)],
            }

        wdram = {"k": wk, "v": wv, "r": wr}
        prev_tok = None
        for rep in range(reps):
            prev_tok = _emit_body(nc, tc, cfg, rep, tm_sb, lam_sb, ub_sb,
                                  a2a, xs, wdram, wo, out, psum,
                                  _collective, tokp, prev_tok, ablate)

    nc.finalize()
    return nc


def _make_token(nc, tokp, osts, R):
    tok = tokp.tile([1, 8], F32, tag="tok", name=R + "tok")
    for i, o in enumerate(osts):
        nc.vector.tensor_copy(tok[0:1, 2 * (i % 4):2 * (i % 4) + 2],
                              o[0:1, 0:2])
    return tok


def _emit_body(nc, tc, cfg, rep, tm_sb, lam_sb, ub_sb, a2a,
               xs, wdram, wo, out, psum, _collective, tokp, prev_tok,
               ablate=None):
    T, NC, TSL, TH, NH = cfg.T, cfg.NC, cfg.TSL, cfg.TH, cfg.NH
    XW = TSL + P
    R = f"r{rep}_"

    with ExitStack() as body:
        # pools that span multiple phases (stack-allocated: LIFO only)
        srp = body.enter_context(tc.tile_pool(name=R + "srp", bufs=1))
        mixrp = body.enter_context(tc.tile_pool(name=R + "mixr", bufs=1))
        scanp = body.enter_context(tc.tile_pool(name=R + "scanp", bufs=2))
        carryp = body.enter_context(tc.tile_pool(name=R + "carryp", bufs=2))
        scan1p = body.enter_context(tc.tile_pool(name=R + "scan1p", bufs=1))
        atbp = body.enter_context(tc.tile_pool(name=R + "atbp", bufs=1))

        srT = [srp.tile([P, TSL], BF16, tag=f"sr{dt}", name=R + f"sr{dt}")
               for dt in range(cfg.NDT)]
        mxr_big = mixrp.tile([P, cfg.NKT * TSL], F8, tag="mxr",
                             name=R + "mxr")
        mixes = {"r": [mxr_big[:, kt * TSL: (kt + 1) * TSL]
                       for kt in range(cfg.NKT)]}
        atb = atbp.tile([P, cfg.NKT2 * TSL], BF16, tag="atb", name=R + "atb")

        with tc.tile_pool(name=R + "mixkv", bufs=1) as mixkvp, \
             tc.tile_pool(name=R + "wstp", bufs=3) as wstp, \
             tc.tile_pool(name=R + "slabp", bufs=3) as slabp, \
             tc.tile_pool(name=R + "q0p", bufs=1) as q0p:
            # ---- step 0: transpose x (chunked), all time-mixes ------------
            with tc.tile_pool(name=R + "xtp", bufs=1) as xtp:
                xtrb = xtp.tile([P, cfg.NKT * XW], BF16, tag="xtrb",
                                name=R + "xtrb")
                if rep > 0:
                    nc.vector.tensor_copy(xtrb[0:1, 0:8], prev_tok[0:1, 0:8])
                for q in range(4):
                    kt0 = 4 * q
                    b = xtrb[:, XW * kt0: XW * (kt0 + 4)]
                    out3 = bass.AP(b.tensor, b.offset,
                                   [b.ap[0], [XW, 4], [1, XW]])
                    nc.sync.dma_start(out3, xs[:, P * kt0: P * (kt0 + 4)],
                                      transpose=True)
                for name in ("k", "v"):
                    mixes[name] = [mixkvp.tile([P, TSL], BF16,
                                               tag=f"mx{name}{kt}",
                                               name=R + f"mx{name}{kt}")
                                   for kt in range(cfg.NKT)]
                with tc.tile_pool(name=R + "dtp", bufs=cfg.NKT) as dtp:
                    dts = []
                    for kt in range(cfg.NKT):
                        xm = xtrb[:, XW * kt + P: XW * (kt + 1)]
                        xx = xtrb[:, XW * kt + P - 1: XW * (kt + 1) - 1]
                        d = dtp.tile([P, TSL], BF16, tag="d", name=R + f"d{kt}")
                        nc.vector.tensor_sub(d[:], xm, xx)
                        nc.vector.scalar_tensor_tensor(
                            mixes["k"][kt][:], d[:], tm_sb["k"][:, kt:kt + 1],
                            xx, op0=AL.mult, op1=AL.add)
                        dts.append((d, xx))
                    for name in ("v", "r"):
                        for kt in range(cfg.NKT):
                            d, xx = dts[kt]
                            mx = mixes[name][kt]
                            if not isinstance(mx, bass.AP):
                                mx = mx[:]
                            nc.vector.scalar_tensor_tensor(
                                mx, d[:],
                                tm_sb[name][:, kt:kt + 1], xx,
                                op0=AL.mult, op1=AL.add)
            # xtrb + d pools closed (space reusable once mixes have run)

            # ---- step 1: k/v projections, strip-interleaved ---------------
            def proj_group(name, dt, wtsl):
                pt_ = psum.tile([P, TSL], F32, tag="pp",
                                name=R + f"ps_{name}_{dt}")
                s4 = dt % 4
                for kt in range(cfg.NKT):
                    nc.tensor.matmul(
                        pt_[:], wtsl(kt, s4),
                        mixes[name][kt][:, :],
                        start=(kt == 0), stop=(kt == cfg.NKT - 1))
                if name == "r":
                    nc.scalar.copy(srT[dt][:], pt_[:])
                else:
                    slab = slabp.tile([P, TSL], BF16, tag="slab",
                                      name=R + f"sl_{name}_{dt}")
                    nc.scalar.copy(slab[:], pt_[:])
                    h, j = dt // 8, dt % 8
                    nc.sync.dma_start(
                        a2a[name]["in"][h][P * j: P * (j + 1), :], slab[:])

            order = [(name, g) for g in range(cfg.NG)
                     for name in ("k", "v")]
            loaded = {}

            def ensure(i):
                if 0 <= i < len(order) and i not in loaded:
                    name, g = order[i]
                    if i == 0:
                        qs = []
                        for q in range(4):
                            qt = q0p.tile([P, 2048], BF16, tag=f"q{q}",
                                          name=R + f"wq{q}")
                            nc.scalar.dma_start(
                                qt[:], wdram["k"][0:P,
                                                  2048 * q: 2048 * (q + 1)])
                            qs.append(qt)
                        loaded[i] = ("q", qs)
                    else:
                        wt = wstp.tile([P, cfg.NKT * 512], BF16, tag="wst",
                                       name=R + f"w_{name}_{g}")
                        nc.scalar.dma_start(
                            wt[:], wdram[name][P * g: P * (g + 1), :])
                        loaded[i] = ("s", wt)

            def mk_wtsl(entry):
                kind, w = entry
                if kind == "q":
                    return lambda kt, s4: w[kt // 4][
                        :, (kt % 4) * 512 + 128 * s4:
                        (kt % 4) * 512 + 128 * (s4 + 1)]
                return lambda kt, s4: w[
                    :, kt * 512 + 128 * s4: kt * 512 + 128 * (s4 + 1)]

            for i, (name, g) in enumerate(order):
                ensure(i), ensure(i + 1), ensure(i + 2)
                wtsl = mk_wtsl(loaded[i])
                for dt in range(4 * g, 4 * g + 4):
                    proj_group(name, dt, wtsl)
                if dt in (7, 15):
                    h = dt // 8
                    _collective([a2a[name]["in"][h][:].opt()],
                                [a2a[name]["out"][h][:].opt()])

            # ---- step 2: r strip loads (SP queue; wstp rotation) ----------
            rstrips = {}
            for g in range(cfg.NG):
                wt = wstp.tile([P, cfg.NKT * 512], F8, tag="wst",
                               name=R + f"w_r_{g}")
                nc.sync.dma_start(wt[:], wdram["r"][P * g: P * (g + 1), :])
                rstrips[g] = wt

            # wo prefetch on the SP queue (transfers run during the scan)
            wotile = [None] * cfg.NOT

            def load_wo(wop):
                for nt in range(cfg.NOT):
                    wot = wop.tile([P, cfg.NKT2 * 512], BF16, tag="wo",
                                   name=R + f"wo_{nt}")
                    nc.sync.dma_start(wot[:], wo[P * nt: P * (nt + 1), :])
                    wotile[nt] = wot

            # ---- step 3: phase B — WKV scan pt0 then pt1 ------------------
            # core's ptile p == global ptile 8p + rank, from A2A half p.
            for p in ([] if ablate == "A" else range(cfg.NCT)):
                lam_b = _bcast(lam_sb[:, p:p + 1], TH)
                ub_ap = ub_sb[:, p:p + 1]
                prevP = prevQ = None
                for c in range(NH):
                    sfx = f"_{p}_{c}"
                    BPC = TH // TSL        # token blocks per chunk
                    j0 = c * BPC

                    def rb(name):
                        t = scanp.tile([P, TH], BF16, tag=f"rb{name}",
                                       name=R + name + sfx)
                        s = a2a[name]["out"][p][:]
                        src = bass.AP(s.tensor, s.offset + j0 * P * TSL,
                                      [[TSL, P], [P * TSL, BPC], [1, TSL]])
                        dst = bass.AP(t[:, :].tensor, t[:, :].offset,
                                      [t[:, :].ap[0], [TSL, BPC], [1, TSL]])
                        nc.scalar.dma_start(dst, src)
                        return t

                    kc, vc = rb("k"), rb("v")
                    ek = scanp.tile([P, TH], BF16, tag="ek",
                                    name=R + "ek" + sfx)
                    nc.scalar.activation(ek[:], kc[:], ACTF.Exp)
                    eku = scanp.tile([P, TH], BF16, tag="eku",
                                     name=R + "eku" + sfx)
                    nc.scalar.activation(eku[:], kc[:], ACTF.Exp, bias=ub_ap)
                    ekv = scan1p.tile([P, TH], BF16, tag="ekv",
                                      name=R + "ekv" + sfx)
                    nc.gpsimd.tensor_mul(ekv[:], ek[:], vc[:])
                    ekuv = scan1p.tile([P, TH], BF16, tag="ekuv",
                                       name=R + "ekuv" + sfx)
                    nc.gpsimd.tensor_mul(ekuv[:], eku[:], vc[:])

                    Pst = carryp.tile([P, TH + 1], BF16, tag="Pst",
                                      name=R + "P" + sfx)
                    Qst = carryp.tile([P, TH + 1], BF16, tag="Qst",
                                      name=R + "Q" + sfx)
                    if c == 0:
                        nc.gpsimd.memset(Pst[:, 0:1], 0.0)
                        nc.gpsimd.memset(Qst[:, 0:1], 0.0)
                    else:
                        nc.gpsimd.tensor_copy(Pst[:, 0:1], prevP[:, TH:TH + 1])
                        nc.gpsimd.tensor_copy(Qst[:, 0:1], prevQ[:, TH:TH + 1])
                    nc.vector.tensor_tensor_scan(
                        Pst[:, 1:TH + 1], lam_b, ekv[:], Pst[:, 0:1],
                        op0=AL.mult, op1=AL.add)
                    nc.vector.tensor_tensor_scan(
                        Qst[:, 1:TH + 1], lam_b, ek[:], Qst[:, 0:1],
                        op0=AL.mult, op1=AL.add)

                    num = ekuv
                    nc.vector.tensor_add(num[:], ekuv[:], Pst[:, 0:TH])
                    den = scan1p.tile([P, TH], F32, tag="den",
                                      name=R + "den" + sfx)
                    nc.vector.tensor_add(den[:], eku[:], Qst[:, 0:TH])
                    nc.vector.reciprocal_approx_fast(den[:], den[:])
                    y = kc    # kc dead once both exps have run; reuse as y
                    nc.vector.tensor_mul(y[:], num[:], den[:])

                    d = a2a["y"]["in"][p][:]
                    dst = bass.AP(d.tensor, d.offset + j0 * P * TSL,
                                  [[TSL, P], [P * TSL, BPC], [1, TSL]])
                    src = bass.AP(y[:, :].tensor, y[:, :].offset,
                                  [y[:, :].ap[0], [TSL, BPC], [1, TSL]])
                    nc.sync.dma_start(dst, src)
                    prevP, prevQ = Pst, Qst

            # ---- step 4: r projections (fp8 DoubleRow) + sigmoids ---------
            for g in range(cfg.NG):
                for dt in range(4 * g, 4 * g + 4):
                    pt_ = psum.tile([P, TSL], F32, tag="pp",
                                    name=R + f"ps_r_{dt}")
                    wt = rstrips[g][:, :]
                    mb = mxr_big[:, :]
                    s4 = dt % 4
                    for j in range(cfg.NKT // 2):
                        stat = bass.AP(wt.tensor,
                                       wt.offset + j * 1024 + s4 * 256,
                                       [wt.ap[0], [128, 2], [1, 128]])
                        mov = bass.AP(mb.tensor, mb.offset + 2 * j * TSL,
                                      [mb.ap[0], [TSL, 2], [1, TSL]])
                        nc.tensor.matmul(
                            pt_[:], stat, mov,
                            start=(j == 0), stop=(j == cfg.NKT // 2 - 1),
                            perf_mode=mybir.MatmulPerfMode.DoubleRow)
                    nc.scalar.copy(srT[dt][:], pt_[:])
                    nc.scalar.activation(srT[dt][:], srT[dt][:], ACTF.Sigmoid)
        # mixkv + wstp + slabp closed

        # ---- step 5: wo loads (gpsimd queue), y A2As + atb assembly -------
        with tc.tile_pool(name=R + "wop", bufs=4) as wop, \
             tc.tile_pool(name=R + "partp", bufs=16) as partp, \
             tc.tile_pool(name=R + "ostl", bufs=6) as ostl:
            load_wo(wop)

            def atb_half(h):
                for kt2 in range(8 * h, 8 * h + 8):
                    j = kt2 % 8
                    sl = atb[:, kt2 * TSL: (kt2 + 1) * TSL]
                    nc.gpsimd.dma_start(
                        sl, a2a["y"]["out"][h][P * j: P * (j + 1), :])

            _collective([a2a["y"]["in"][0][:].opt()],
                        [a2a["y"]["out"][0][:].opt()])
            atb_half(0)
            _collective([a2a["y"]["in"][1][:].opt()],
                        [a2a["y"]["out"][1][:].opt()])
            atb_half(1)

            # ---- step 6: phase C — sr*y muls (DVE) + output matmul --------
            for kt2 in range(cfg.NKT2):
                sl = atb[:, kt2 * TSL: (kt2 + 1) * TSL]
                nc.vector.tensor_mul(sl, sl, srT[kt2][:])

            if ablate == "B":
                lastc = []
                for mt in range(cfg.NMT):
                    oc = ostl.tile([P, 512], F32, tag="oc",
                                   name=R + f"abl_{mt}")
                    nc.scalar.copy(oc[:], atb[:, mt * 2048: mt * 2048 + 512])
                    nc.sync.dma_start(out[P * mt: P * (mt + 1), 0:512], oc[:])
                    lastc.append(oc)
                return _make_token(nc, tokp, lastc, R)

            # h0 contraction first (all four nt strips), bf16 partial
            # drains; h1 sweeps start once the second y half lands.
            parts = {}
            for rnd in range(cfg.NOT // 2):
                nts = (2 * rnd, 2 * rnd + 1)
                pts = {(mt_, i_): psum.tile([P, 512], F32, tag="pp",
                                            name=R + f"pa_{rnd}_{mt_}_{i_}")
                       for mt_ in range(cfg.NMT) for i_ in range(2)}
                for kt2 in range(8):
                    for mt in range(cfg.NMT):
                        lhsT = atb[:, kt2 * TSL + P * mt:
                                   kt2 * TSL + P * (mt + 1)]
                        for i_ in range(2):
                            nc.tensor.matmul(
                                pts[(mt, i_)][:], lhsT,
                                wotile[nts[i_]][:, 512 * kt2: 512 * (kt2 + 1)],
                                start=(kt2 == 0), stop=(kt2 == 7))
                for mt in range(cfg.NMT):
                    for i_ in range(2):
                        pb = partp.tile([P, 512], BF16, tag="part",
                                        name=R + f"pb_{rnd}_{mt}_{i_}")
                        nc.scalar.copy(pb[:], pts[(mt, i_)][:])
                        parts[(mt, nts[i_])] = pb

            lastc = []
            for rnd in range(cfg.NOT // 2):
                nts = (2 * rnd, 2 * rnd + 1)
                pts = {(mt_, i_): psum.tile([P, 512], F32, tag="pp",
                                            name=R + f"po_{rnd}_{mt_}_{i_}")
                       for mt_ in range(cfg.NMT) for i_ in range(2)}
                for kt2 in range(8, cfg.NKT2):
                    for mt in range(cfg.NMT):
                        lhsT = atb[:, kt2 * TSL + P * mt:
                                   kt2 * TSL + P * (mt + 1)]
                        for i_ in range(2):
                            nc.tensor.matmul(
                                pts[(mt, i_)][:], lhsT,
                                wotile[nts[i_]][:, 512 * kt2: 512 * (kt2 + 1)],
                                start=(kt2 == 8), stop=(kt2 == cfg.NKT2 - 1))
                for mt in range(cfg.NMT):
                    for i_ in range(2):
                        nt = nts[i_]
                        oc = ostl.tile([P, 512], F32, tag="oc",
                                       name=R + f"oc_{rnd}_{mt}_{i_}")
                        nc.vector.tensor_add(oc[:], parts[(mt, nt)][:],
                                             pts[(mt, i_)][:])
                        nc.sync.dma_start(
                            out[P * mt: P * (mt + 1),
                                512 * nt: 512 * (nt + 1)],
                            oc[:])
                        if rnd == cfg.NOT // 2 - 1 and i_ == 1:
                            lastc.append(oc)
            tok = _make_token(nc, tokp, lastc, R)
    return tok


# ------------------------------------------------------------------------
# host side
# ------------------------------------------------------------------------

_CACHE = {}


def _get_nc(cfg: Cfg):
    key = (cfg.T, cfg.NE, cfg.DA, cfg.NC, cfg.TH)
    if key not in _CACHE:
        _CACHE[key] = build_kernel(cfg)
    return _CACHE[key]


def make_in_maps(cfg: Cfg, x, time_first, time_decay, time_mix_k, time_mix_v,
                 time_mix_r, W_key, W_value, W_receptance, W_output):
    T, NE, DA, NC = cfg.T, cfg.NE, cfg.DA, cfg.NC
    TSL = cfg.TSL
    bf = ml_dtypes.bfloat16

    x = np.asarray(x, np.float32)
    xpad = np.zeros((P + T, NE), bf)
    xpad[P:] = x.astype(bf)

    def tile_w(w, nkt, ng):
        w = np.asarray(w, np.float32).astype(bf)
        return np.ascontiguousarray(
            w.reshape(nkt, P, ng, 512).transpose(2, 1, 0, 3)
            .reshape(ng * P, nkt * 512))

    wk16 = tile_w(W_key, cfg.NKT, cfg.NG)
    wv16 = tile_w(W_value, cfg.NKT, cfg.NG)
    wo16 = tile_w(W_output, cfg.NKT2, cfg.NOT)
    # r weights: fp8 e4m3 packed for DoubleRow —
    # [g*P+p, j*1024 + c4*256 + i*128 + m] = Wr[128*(2j+i)+p, 512g+128c4+m]
    f8np = mybir.dt.np(F8)
    wr4 = np.asarray(W_receptance, np.float32).astype(f8np) \
        .reshape(cfg.NKT // 2, 2, P, cfg.NG, 4, 128)
    wr8 = np.ascontiguousarray(
        wr4.transpose(3, 2, 0, 4, 1, 5).reshape(cfg.NG * P, cfg.NKT * 512))

    def col_fold(v, n_t):  # [n_t*P] -> [P, n_t]
        return np.ascontiguousarray(
            np.asarray(v, np.float64).reshape(-1)[: n_t * P]
            .reshape(n_t, P).T.astype(np.float32))

    tmk_a = col_fold(time_mix_k, cfg.NKT)
    tmv_a = col_fold(time_mix_v, cfg.NKT)
    tmr_a = col_fold(time_mix_r, cfg.NKT)

    td = np.asarray(time_decay, np.float64).reshape(-1)
    lam_full = np.exp(-np.exp(td)).astype(np.float32)
    ub_full = np.asarray(time_first, np.float32).reshape(-1)

    in_maps = []
    for i in range(NC):
        xsl = np.ascontiguousarray(xpad[TSL * i: TSL * i + TSL + P, :])
        # core i owns global channel ptiles {i, i+8}
        lam_i = np.stack([lam_full[P * (8 * p + i): P * (8 * p + i + 1)]
                          for p in range(cfg.NCT)], axis=1)
        ub_i = np.stack([ub_full[P * (8 * p + i): P * (8 * p + i + 1)]
                         for p in range(cfg.NCT)], axis=1)
        in_maps.append({
            "xs": xsl, "wk": wk16, "wv": wv16, "wr": wr8, "wo": wo16,
            "tmk": tmk_a, "tmv": tmv_a, "tmr": tmr_a,
            "lam": np.ascontiguousarray(lam_i),
            "ub": np.ascontiguousarray(ub_i),
        })
    return in_maps


def kernel(x, time_first, time_decay, time_mix_k, time_mix_v, time_mix_r,
           W_key, W_value, W_receptance, W_output, _trace=False):
    cfg = Cfg(T=int(np.asarray(x).shape[0]), NE=int(np.asarray(x).shape[1]),
              DA=int(np.asarray(time_decay).reshape(-1).shape[0]), NC=8)
    nc = _get_nc(cfg)
    in_maps = make_in_maps(cfg, x, time_first, time_decay, time_mix_k,
                           time_mix_v, time_mix_r, W_key, W_value,
                           W_receptance, W_output)
    res = run_bass_kernel_spmd(nc, in_maps, core_ids=list(range(cfg.NC)),
                               trace=_trace)
    outp = np.concatenate([res.results[i]["out"] for i in range(cfg.NC)], axis=0)
    out_final = outp.astype(np.float32)
    if _trace:
        return out_final, res
    return out_final
